# revision 27
# baseline (speedup 1.0000x reference)
"""Chamfer distance loss on 8 TRN2 NeuronCores.

Strategy (data-parallel over batch, 4 batches per core):
  - Host gathers the 2048-point subsets p1 = points1[:, idx1], p2 = points2[:, idx2].
  - Squared pairwise distances (minus the per-query norm n1, constant per row
    and hence irrelevant to the row-min) are computed on TensorE as a K=21
    augmented matmul: P[s,t] = n2_t - 2*p1_s.p2_t, every f32 quantity split
    into 3 bf16 components so all products are exact in the PE array.
  - Exact windowed NN: both point sets sorted along x; per 128-query chunk a
    provably sufficient target window is derived from host-side NN upper
    bounds (u_s = best distance among rank neighbors along x/morton/y/z; the
    true NN must satisfy |x_t - x_s| <= u_s).
  - SPMD-safe slotting: each core sorts its 128 (batch, dir, chunk) units by
    window width; slot k of the shared program uses width SCHED[k] >= every
    core's k-th widest unit.
  - Three parallel drain lanes, greedily balanced per slot:
      V: DVE tensor_tensor_reduce(min) straight from PSUM, with both input
         APs inside a single PSUM bank (multi-bank DVE PSUM APs hit a HW
         slow path). One f32 partial per 512-block.
      A: ScalarE (Activation) Exp softmin: accum_s = sum_t exp(-k*(P_st -
         b_s)) with per-partition bias b_s = k*(u_s^2 - n1_s); host recovers
         min via u^2 - log(accum)/k. Bias error ~log(ties)/k; window
         truncation is damped by the padded window. Overflow (accum >= 1e30)
         is detected host-side and those queries recomputed exactly.
      P: Pool (gpsimd) tensor_tensor(min) single-bank folds PSUM -> f32
         SBUF, finished by one DVE TTR over the folded buffer.
  - Host does the final add-n1 / sqrt / means over the 8 cores' outputs.
"""

import os
import numpy as np
import ml_dtypes

import concourse.bass as bass
from concourse import bacc
import concourse.tile as tile
from concourse import mybir
from concourse.bass_utils import run_bass_kernel_spmd

BF16 = ml_dtypes.bfloat16

B = 32              # global batch
S = 2048            # sampled points per cloud
N_CORES = 8
B_LOC = B // N_CORES
N_CHUNKS = S // 128
N_UNITS = B_LOC * 2 * N_CHUNKS  # 128 slots per core
KC = 32             # rank-neighbor candidates per sort axis
QUANT = 64          # window width quantum
WMIN = 64
N_GROUPS = 4        # PE row groups
KAUG = 21           # augmented matmul contraction rows
SOFTMIN_K = 4000.0
ACAP = int(os.environ.get("CHAMFER_ACAP", "1024"))
PSA_BUFS = int(os.environ.get("CHAMFER_PSA", "2"))
PSB_BUFS = int(os.environ.get("CHAMFER_PSB", "4"))
ORDER = os.environ.get("CHAMFER_ORDER", "desc")  # desc | asc | zip
AFUDGE = float(os.environ.get("CHAMFER_AFUDGE", "1.0"))
VFUDGE = float(os.environ.get("CHAMFER_VFUDGE", "1.0"))
ACC_CLIP = 1.0e30   # host falls back to exact recompute past this


# ---------------------------------------------------------------- host math

def _split3(x):
    h = x.astype(BF16).astype(np.float64)
    m = (x - h).astype(BF16).astype(np.float64)
    l = (x - h - m).astype(BF16).astype(np.float64)
    return h, m, l


def _build_aug(a, b):
    """a, b: (S, 3) float64 point sets (query side a, target side b).
    Returns A, Bm: (21, S) bf16 with A[:, s] . B[:, t] == |a_s - b_t|^2 -
    |a_s|^2 up to ~1e-6."""
    ah, am, al = _split3(a)
    bh, bm, bl = _split3(b)
    n2h, n2m, n2l = _split3((b * b).sum(1))
    A = np.zeros((KAUG, a.shape[0]))
    Bm = np.zeros((KAUG, b.shape[0]))
    pairs = [(ah, bh), (ah, bm), (am, bh), (ah, bl), (al, bh), (am, bm)]
    for k, (x, y) in enumerate(pairs):
        A[3 * k:3 * k + 3] = (-2.0 * x).T
        Bm[3 * k:3 * k + 3] = y.T
    A[18:21] = 1.0
    Bm[18], Bm[19], Bm[20] = n2h, n2m, n2l
    return A.astype(BF16), Bm.astype(BF16)


def _morton_key(p):
    q = np.clip(((p + 4.0) / 8.0 * 1024).astype(np.int64), 0, 1023)

    def spread(x):
        x = (x | (x << 16)) & 0x030000FF
        x = (x | (x << 8)) & 0x0300F00F
        x = (x | (x << 4)) & 0x030C30C3
        x = (x | (x << 2)) & 0x09249249
        return x

    return spread(q[:, 0]) | (spread(q[:, 1]) << 1) | (spread(q[:, 2]) << 2)


def _rank_candidate_u2(q, keyq, t, keyt):
    """Best squared distance among 2*KC rank neighbors of each query under
    the given sort key."""
    to = np.argsort(keyt, kind="stable")
    ts = t[to]
    pos = np.searchsorted(keyt[to], keyq).clip(0, S - 1)
    idx = (pos[:, None] + np.arange(-KC, KC)[None, :]).clip(0, S - 1)
    return ((q[:, None, :] - ts[idx]) ** 2).sum(-1).min(1)


def _unit_windows(q, t):
    """q, t: (S, 3) float64, both sorted by x. Returns (wins, u2) where wins
    is the per-chunk (lo, width) rank window guaranteed to contain every
    query's true NN and u2 the per-query squared NN upper bound."""
    u2 = _rank_candidate_u2(q, q[:, 0], t, t[:, 0])
    u2 = np.minimum(u2, _rank_candidate_u2(q, _morton_key(q), t, _morton_key(t)))
    u2 = np.minimum(u2, _rank_candidate_u2(q, q[:, 1], t, t[:, 1]))
    u2 = np.minimum(u2, _rank_candidate_u2(q, q[:, 2], t, t[:, 2]))
    u = np.sqrt(u2) * (1 + 1e-9)
    lo_x = q[:, 0] - u
    hi_x = q[:, 0] + u
    wins = []
    for ch in range(N_CHUNKS):
        sl = slice(ch * 128, ch * 128 + 128)
        lo = int(np.searchsorted(t[:, 0], lo_x[sl].min(), side="left"))
        hi = int(np.searchsorted(t[:, 0], hi_x[sl].max(), side="right"))
        wins.append((lo, hi - lo))
    return wins, u2


def _prepare(points1, points2, idx1, idx2):
    g1 = np.asarray(points1)[:, np.asarray(idx1)].astype(np.float64)
    g2 = np.asarray(points2)[:, np.asarray(idx2)].astype(np.float64)
    cores = []
    widths = np.zeros((N_CORES, N_UNITS), dtype=np.int64)
    for core in range(N_CORES):
        augs = []    # per bl: (A1s, B2s) for dir 0 and its swap for dir 1
        pts = []     # per bl: (a, c) sorted point sets
        units = []   # (w, lo, bl, dr, ch)
        u2s = {}     # (bl, dr) -> per-query u2
        n1s = {}     # (bl, dr) -> per-query |q|^2
        for bl in range(B_LOC):
            b = core * B_LOC + bl
            a = g1[b][np.argsort(g1[b][:, 0], kind="stable")]
            c = g2[b][np.argsort(g2[b][:, 0], kind="stable")]
            pts.append((a, c))
            augs.append((_build_aug(a, c), _build_aug(c, a)))
            for dr, (q, t) in enumerate(((a, c), (c, a))):
                wins, u2 = _unit_windows(q, t)
                u2s[(bl, dr)] = u2
                n1s[(bl, dr)] = (q * q).sum(1)
                for ch, (lo, w) in enumerate(wins):
                    units.append((w, lo, bl, dr, ch))
        units.sort(key=lambda u: -u[0])
        widths[core] = [u[0] for u in units]
        cores.append({"augs": augs, "pts": pts, "units": units,
                      "u2s": u2s, "n1s": n1s})
    return cores, widths


def _schedule(widths):
    need = widths.max(axis=0)
    sched = (np.ceil(np.maximum(need, WMIN) / QUANT).astype(np.int64) * QUANT).clip(max=S)
    return [int(w) for w in sched]


# ------------------------------------------------------- lane cost model

def _blocks(W, cap=512):
    out = []
    j = 0
    while j < W:
        n = min(cap, W - j)
        out.append((j, n))
        j += n
    return out


def _ttr_cost(pair):
    # DVE TTR (SBUF ins): pair elements per input AP
    return pair * 1.042 + 135


def _lane_costs(W):
    """Returns dict lane -> (dve_ns, act_ns, pool_ns)."""
    v = VFUDGE * sum(n * 1.042 + 135 for _, n in _blocks(W))
    a = AFUDGE * sum(n * 0.833 + 373 for _, n in _blocks(W, cap=ACAP))
    p_pool = sum(n * 1.389 + 131 for _, n in _blocks(W))
    p_dve = _ttr_cost(min(W, 512) // 2)
    return {"V": (v, 0.0, 0.0), "A": (0.0, a, 0.0), "P": (p_dve, 0.0, p_pool)}


def _make_batches(sched):
    """Groups of consecutive same-width slots packed into one PSUM bank
    (m*w <= 512) so one segmented DVE reduce drains m slots. Returns list of
    (k0, m, w)."""
    batches = []
    i = 0
    n = len(sched)
    nobatch = os.environ.get("CHAMFER_NOBATCH", "1") == "1"
    while i < n:
        w = sched[i]
        m = 1
        if w <= 256 and not nobatch:
            while (i + m < n and sched[i + m] == w and (m + 1) * w <= 512
                   and m < 8):
                m += 1
        batches.append((i, m, w))
        i += m
    return batches


def _batch_costs(m, w):
    """lane -> (dve, act, pool) for a batch of m equal slots of width w."""
    if m > 1:
        v = VFUDGE * (m * w * 1.042 + 135)
    else:
        v = VFUDGE * sum(n * 1.042 + 135 for _, n in _blocks(w))
    d, a, p = 0.0, 0.0, 0.0
    for _ in range(m):
        _, sa, _ = _lane_costs(w)["A"]
        pd, _, pp = _lane_costs(w)["P"]
        a += sa
        d += pd
        p += pp
    return {"V": (v, 0.0, 0.0), "A": (0.0, a, 0.0), "P": (d, 0.0, p)}


def _plan_lanes(sched):
    forced = os.environ.get("CHAMFER_LANES", "")
    if forced in ("V", "A", "P"):
        return [forced] * len(sched)
    kinds = tuple(forced) if forced else ("V",)
    batches = _make_batches(sched)
    lane_of = {}
    tot = {"D": 0.0, "A": 0.0, "P": 0.0}

    def costs_of(b, kind):
        d, a, p = _batch_costs(b[1], b[2])[kind]
        return {"D": d, "A": a, "P": p}

    for b in batches:
        best = None
        for kind in kinds:
            if kind == "P" and b[2] < 128:
                continue
            c = costs_of(b, kind)
            mk = max(tot[e] + c[e] for e in tot)
            if best is None or mk < best[0]:
                best = (mk, kind, c)
        lane_of[b[0]] = best[1]
        for e in tot:
            tot[e] += best[2][e]

    # local search: move batches off the max-loaded engine while it helps
    for _ in range(400):
        mk0 = max(tot.values())
        improved = False
        for b in batches:
            cur = lane_of[b[0]]
            cc = costs_of(b, cur)
            for kind in kinds:
                if kind == cur or (kind == "P" and b[2] < 128):
                    continue
                cn = costs_of(b, kind)
                trial = {e: tot[e] - cc[e] + cn[e] for e in tot}
                if max(trial.values()) < mk0 - 1e-9:
                    lane_of[b[0]] = kind
                    tot = trial
                    mk0 = max(tot.values())
                    improved = True
                    break
            if improved:
                break
        if not improved:
            break

    lanes = [None] * len(sched)
    for (k0, m, W) in batches:
        for i in range(k0, k0 + m):
            lanes[i] = lane_of[k0]
    return lanes


# ------------------------------------------------------------- device build

def _program_batches(sched):
    """Batches in program order: a few mid-width batches open the kernel (so
    the small first wb DMA piece unblocks the PE quickly), then the rest in
    rank (width-descending) order."""
    batches = _make_batches(sched)
    openers = []
    rest = []
    for b in batches:
        if len(openers) < 3 and 320 <= b[2] <= 768 and b[0] >= 4:
            openers.append(b)
        else:
            rest.append(b)
    return openers + rest


def _slot_layout(sched):
    """Program-position-derived layout. Returns (off, gc, g_of, blk_of):
    slot k's wb columns live at off[k] of PE row-group g_of[k], its query
    block at tq column block blk_of[k]. Groups and tq blocks follow program
    order so both input DMAs stream in consumption order."""
    n = len(sched)
    off = [0] * n
    g_of = [0] * n
    blk_of = [0] * n
    gsum = [0] * N_GROUPS
    i = 0
    for (k0, m, w) in _program_batches(sched):
        for k in range(k0, k0 + m):
            g = i % N_GROUPS
            g_of[k] = g
            blk_of[k] = i // N_GROUPS
            off[k] = gsum[g]
            gsum[g] += sched[k]
            i += 1
    return off, max(gsum), g_of, blk_of


def _build_nc_v3(sched, lanes, reps=1):
    off, gc, g_of, blk_of = _slot_layout(sched)
    n_tq_cols = ((N_UNITS + N_GROUPS - 1) // N_GROUPS) * 128
    nc = bacc.Bacc()
    tq_d = nc.declare_dram_parameter("tq", [128, n_tq_cols], mybir.dt.bfloat16, isOutput=False)
    wb_d = nc.declare_dram_parameter("wb", [128, gc], mybir.dt.bfloat16, isOutput=False)
    bias_d = nc.declare_dram_parameter("bias", [128, N_UNITS], mybir.dt.float32, isOutput=False)
    out_d = nc.declare_dram_parameter("out", [128, 5 * N_UNITS], mybir.dt.float32, isOutput=True)

    X = mybir.AxisListType.X
    MIN = mybir.AluOpType.min
    EXP = mybir.ActivationFunctionType.Exp

    with tile.TileContext(nc) as tc:
        with (
            tc.tile_pool(name="inp", bufs=1) as inp,
            tc.tile_pool(name="sb", bufs=1) as sbp,
            tc.tile_pool(name="fold", bufs=2) as fbp,
            tc.tile_pool(name="psA", bufs=PSA_BUFS, space="PSUM") as psA,
            tc.tile_pool(name="psB", bufs=PSB_BUFS, space="PSUM") as psB,
        ):
            tq = inp.tile([128, n_tq_cols], mybir.dt.bfloat16)
            wb = inp.tile([128, gc], mybir.dt.bfloat16)
            biases = inp.tile([128, N_UNITS], mybir.dt.float32)
            # DMA pieces ordered by first consumption: a small tq piece and a
            # small first wb piece unblock slot 0, biases unblock the first
            # A-lane Exp, the rest streams in under compute. Descriptor
            # generation (HWDGE, ~630ns each) is spread across three engine
            # queues so the first transfers start early.
            cuts = [0]
            for c in (512, 1536, 3584, 7680):
                if c < gc:
                    cuts.append(c)
            cuts.append(gc)
            dq = nc.sync if os.environ.get("CHAMFER_SYNCDMA", "1") == "1" else nc.scalar
            dq.dma_start(tq[:, 0:128], tq_d[:, 0:128])
            nc.sync.dma_start(wb[:, cuts[0]:cuts[1]], wb_d[:, cuts[0]:cuts[1]])
            dq.dma_start(biases[:], bias_d[:])
            dq.dma_start(tq[:, 128:512], tq_d[:, 128:512])
            nc.sync.dma_start(wb[:, cuts[1]:cuts[2]], wb_d[:, cuts[1]:cuts[2]])
            dq.dma_start(tq[:, 512:n_tq_cols], tq_d[:, 512:n_tq_cols])
            for i in range(2, len(cuts) - 1):
                nc.sync.dma_start(wb[:, cuts[i]:cuts[i + 1]], wb_d[:, cuts[i]:cuts[i + 1]])

            mins = sbp.tile([128, 4 * N_UNITS], mybir.dt.float32)
            nc.vector.memset(mins[:], 3.0e38)
            smins = sbp.tile([128, N_UNITS], mybir.dt.float32)
            nc.vector.memset(smins[:], 3.0e38)
            junkV = sbp.tile([128, 512], mybir.dt.float32)
            junkP = sbp.tile([128, 512], mybir.dt.float32)
            junkE = sbp.tile([128, ACAP], mybir.dt.bfloat16)

            def slot_lhsT_outer(k):
                g, kg = g_of[k], blk_of[k]
                return 32 * g, tq[32 * g:32 * g + KAUG, kg * 128:(kg + 1) * 128]

            def body(_i=None):
                batches = _program_batches(sched)
                slot_lhsT = slot_lhsT_outer

                for (k0, m, Wb) in batches:
                    if lanes[k0] == "V" and m > 1:
                        ps = psB.tile([128, 512], mybir.dt.float32)
                        for i in range(m):
                            k = k0 + i
                            p0, lhsT = slot_lhsT(k)
                            rhs = wb[p0:p0 + KAUG, off[k]: off[k] + Wb]
                            nc.tensor.matmul(ps[:, i * Wb:(i + 1) * Wb], lhsT, rhs,
                                             start=True, stop=True, tile_position=(p0, 0))
                        seg = ps[:, 0:m * Wb].rearrange("p (m w) -> p m w", m=m)
                        nc.vector.tensor_reduce(
                            out=smins[:, k0:k0 + m], in_=seg,
                            axis=X, op=MIN)
                        continue
                    for k in range(k0, k0 + m):
                        _slot_body(k)

            def _slot_body(k):
                    W = sched[k]
                    p0, lhsT = slot_lhsT_outer(k)
                    lane = lanes[k]
                    if lane == "A":
                        npart = 0
                        for cj, cn in _blocks(W, cap=ACAP):
                            ps = psA.tile([128, ACAP], mybir.dt.float32)
                            for j, n in _blocks(cn):
                                rhs = wb[p0:p0 + KAUG, off[k] + cj + j: off[k] + cj + j + n]
                                nc.tensor.matmul(ps[:, j:j + n], lhsT, rhs,
                                                 start=True, stop=True, tile_position=(p0, 0))
                            eout = ps[:, 0:cn] if not os.environ.get("CHAMFER_EXP_SBUF") else junkE[:, 0:cn]
                            nc.scalar.activation(
                                out=eout, in_=ps[:, 0:cn], func=EXP,
                                bias=biases[:, k:k + 1], scale=-SOFTMIN_K,
                                accum_out=mins[:, 4 * k + npart:4 * k + npart + 1])
                            npart += 1
                    elif lane == "V":
                        npart = 0
                        for j, n in _blocks(W):
                            ps = psB.tile([128, 512], mybir.dt.float32)
                            rhs = wb[p0:p0 + KAUG, off[k] + j: off[k] + j + n]
                            nc.tensor.matmul(ps[:, 0:n], lhsT, rhs,
                                             start=True, stop=True, tile_position=(p0, 0))
                            nc.vector.tensor_reduce(
                                out=mins[:, 4 * k + npart:4 * k + npart + 1],
                                in_=ps[:, 0:n], axis=X, op=MIN)
                            npart += 1
                    else:  # "P"
                        # gpsimd min-accumulates 512-blocks into an SBUF strip
                        # (only one PSUM operand per instruction is legal),
                        # then DVE finishes from SBUF.
                        fb = fbp.tile([128, 512], mybir.dt.float32)
                        acc_w = 0
                        for j, n in _blocks(W):
                            ps = psB.tile([128, 512], mybir.dt.float32)
                            rhs = wb[p0:p0 + KAUG, off[k] + j: off[k] + j + n]
                            nc.tensor.matmul(ps[:, 0:n], lhsT, rhs,
                                             start=True, stop=True, tile_position=(p0, 0))
                            if acc_w == 0:
                                nc.gpsimd.tensor_scalar(
                                    out=fb[:, 0:n], in0=ps[:, 0:n],
                                    scalar1=3.0e38, scalar2=None,
                                    op0=MIN)
                                acc_w = n
                            else:
                                nc.gpsimd.tensor_tensor(
                                    out=fb[:, 0:n], in0=ps[:, 0:n], in1=fb[:, 0:n],
                                    op=MIN)
                        q = acc_w // 2
                        nc.vector.tensor_tensor_reduce(
                            out=junkP[:, 0:q], in0=fb[:, 0:q], in1=fb[:, q:2 * q],
                            scale=1.0, scalar=3.0e38, op0=MIN, op1=MIN,
                            accum_out=mins[:, 4 * k:4 * k + 1])

            if reps > 1:
                with tc.For_i(0, reps, 1):
                    body()
            else:
                body()

            half = 2 * N_UNITS
            nc.sync.dma_start(out_d[:, 0:half], mins[:, 0:half])
            nc.sync.dma_start(out_d[:, half:4 * N_UNITS], mins[:, half:])
            nc.sync.dma_start(out_d[:, 4 * N_UNITS:], smins[:])
    if not nc.is_finalized():
        nc.finalize()
    return nc


_NC_CACHE = {}


def _get_nc_v3(sched, lanes, reps=1):
    key = (tuple(sched), tuple(lanes), reps)
    if key not in _NC_CACHE:
        _NC_CACHE[key] = _build_nc_v3(sched, lanes, reps)
    return _NC_CACHE[key]


def _make_in_maps(cores, sched, lanes):
    off, gc, g_of, blk_of = _slot_layout(sched)
    n_tq_cols = ((N_UNITS + N_GROUPS - 1) // N_GROUPS) * 128
    in_maps = []
    for core in range(N_CORES):
        tq = np.zeros((128, n_tq_cols), dtype=BF16)
        wb = np.zeros((128, gc), dtype=BF16)
        bias = np.zeros((128, N_UNITS), dtype=np.float32)
        meta = []
        for k, (w, lo, bl, dr, ch) in enumerate(cores[core]["units"]):
            W = sched[k]
            g, kg = g_of[k], blk_of[k]
            p0 = 32 * g
            aug0, aug1 = cores[core]["augs"][bl]
            qsrc, tsrc = aug0 if dr == 0 else aug1
            tq[p0:p0 + KAUG, kg * 128:(kg + 1) * 128] = qsrc[:, ch * 128:(ch + 1) * 128]
            lo2 = min(max(lo - (W - w) // 2, 0), S - W)
            wb[p0:p0 + KAUG, off[k]:off[k] + W] = tsrc[:, lo2:lo2 + W]
            u2 = cores[core]["u2s"][(bl, dr)][ch * 128:(ch + 1) * 128]
            n1 = cores[core]["n1s"][(bl, dr)][ch * 128:(ch + 1) * 128]
            bias[:, k] = (SOFTMIN_K * (u2 - n1)).astype(np.float32)
            meta.append((bl, dr, ch, lo2))
        in_maps.append({"tq": tq, "wb": wb, "bias": bias})
        cores[core]["meta"] = meta
    return in_maps


def _reduce_outputs_v3(results, cores, sched, lanes):
    total = 0.0
    batched = set()
    for (k0, m, w) in _make_batches(sched):
        if lanes[k0] == "V" and m > 1:
            batched.update(range(k0, k0 + m))
    for core in range(N_CORES):
        raw = np.asarray(results[core]["out"], dtype=np.float64)  # (128, 5*N_UNITS)
        smins = raw[:, 4 * N_UNITS:]
        cd = cores[core]
        for k, (w, lo, bl, dr, ch) in enumerate(cd["units"]):
            u2 = cd["u2s"][(bl, dr)][ch * 128:(ch + 1) * 128]
            n1 = cd["n1s"][(bl, dr)][ch * 128:(ch + 1) * 128]
            parts = raw[:, 4 * k:4 * k + 4]
            if k in batched:
                d2 = smins[:, k] + n1
            elif lanes[k] == "A":
                nchunks = len(_blocks(sched[k], cap=ACAP))
                acc = parts[:, :nchunks].sum(axis=1)
                bad = ~np.isfinite(acc) | (acc >= ACC_CLIP) | (acc <= 0.0)
                with np.errstate(divide="ignore"):
                    d2 = u2 - np.log(np.maximum(acc, 1e-38)) / SOFTMIN_K
                d2 = np.where(bad, np.nan, d2)
            else:
                nblk = len(_blocks(sched[k]))
                d2 = parts[:, :nblk].min(axis=1) + n1
            # exact host fallback for flagged queries
            if np.any(~np.isfinite(d2)):
                q, t = cd["pts"][bl] if dr == 0 else cd["pts"][bl][::-1]
                sel = np.where(~np.isfinite(d2))[0]
                qq = q[ch * 128 + sel]
                d2[sel] = ((qq[:, None, :] - t[None, :, :]) ** 2).sum(-1).min(1)
            total += np.sqrt(np.maximum(d2, 0.0)).sum() / S
    return np.float32(total / B)


def _run(inputs, trace=False, timers=None, reps=None):
    import time as _t
    if reps is None:
        reps = int(os.environ.get("CHAMFER_REPS", "1"))
    t0 = _t.time()
    cores, widths = _prepare(inputs["points1"], inputs["points2"],
                             inputs["idx1"], inputs["idx2"])
    sched = _schedule(widths)
    lanes = _plan_lanes(sched)
    nc = _get_nc_v3(sched, lanes, reps)
    in_maps = _make_in_maps(cores, sched, lanes)
    t1 = _t.time()
    res = run_bass_kernel_spmd(nc, in_maps, core_ids=list(range(N_CORES)),
                               trace=trace)
    t2 = _t.time()
    loss = _reduce_outputs_v3(res.results, cores, sched, lanes)
    if timers is not None:
        timers["prepare_s"] = t1 - t0
        timers["run_s"] = t2 - t1
    return loss, res


def kernel(**inputs):
    loss, _ = _run(inputs, trace=False)
    return loss
